# revision 1
# baseline (speedup 1.0000x reference)
"""DCNv2 deformable PS-RoI pooling on 8 Trainium2 NeuronCores.

Strategy (RoI-data-parallel, 32 rois per core):
  * Host replicates the reference coordinate math exactly in float32 and folds
    bilinear weights, validity masking and the 1/count normalization into a
    per-roi sparse matrix A (bbox_pixels x 49). Each roi touches only a small
    bbox of the 64x64 feature map, so A has ~128-384 rows (padded to 128k).
  * Feature map is transposed to channel-last (B*H*W, C) on host so each pixel
    is a contiguous 1KB channel vector in HBM.
  * Device (SPMD, identical program on 8 cores, per-core data in DRAM inputs):
      - one indirect-DMA gather per column group: patch[p, t, :] =
        Fcl[idx[p, t], :]  -> pixel-on-partition layout
      - per roi: out(c,j) accumulated in PSUM over 128-pixel chunks via
        matmul(lhsT=patch_chunk(128px, 128c), rhs=A_chunk(128px, 49j))
      - PSUM -> SBUF staging -> one contiguous DMA to HBM (c-major scratch
        layout); host undoes the layout permutation while assembling.
"""
import numpy as np

f32 = np.float32
f64 = np.float64

B, C, H, W = 8, 256, 64, 64
N_ROIS, P, S = 256, 7, 4
PART = 7
NJ = P * P  # 49
SCALE = f32(1.0 / 16.0)
TRANS_STD = f32(0.1)
N_CORES = 8
RPC = N_ROIS // N_CORES  # rois per core
N_GROUPS = 6  # gather/compute pipeline groups
GROUP_WEIGHTS = [0.5, 1.4, 1.4, 1.2, 1.0, 0.5, 0.4, 0.3]  # truncated to N_GROUPS
A_SPLIT = False  # upload A per group instead of one shot

_prog_cache = {}


# --------------------------------------------------------------------------
# host math: exact f32 replication of the reference coordinate computation
# --------------------------------------------------------------------------
def _roi_sampling_data(rois, offset):
    rois = np.asarray(rois, dtype=f32)
    offset = np.asarray(offset, dtype=f32)
    batch = rois[:, 0].astype(np.int32)

    roi_sw = np.round(rois[:, 1]) * SCALE - f32(0.5)
    roi_sh = np.round(rois[:, 2]) * SCALE - f32(0.5)
    roi_ew = (np.round(rois[:, 3]) + f32(1.0)) * SCALE - f32(0.5)
    roi_eh = (np.round(rois[:, 4]) + f32(1.0)) * SCALE - f32(0.5)
    roi_w = np.maximum(roi_ew - roi_sw, f32(0.1))
    roi_h = np.maximum(roi_eh - roi_sh, f32(0.1))
    bin_w = roi_w / f32(P)
    bin_h = roi_h / f32(P)
    sub_w = bin_w / f32(S)
    sub_h = bin_h / f32(S)

    ph = np.arange(P, dtype=np.int32)
    pw = np.arange(P, dtype=np.int32)
    part_h = np.clip(
        np.floor(ph.astype(f32) / f32(P) * f32(PART)).astype(np.int32), 0, PART - 1
    )
    part_w = np.clip(
        np.floor(pw.astype(f32) / f32(P) * f32(PART)).astype(np.int32), 0, PART - 1
    )

    tx = offset[:, 0][:, part_h[:, None], part_w[None, :]] * TRANS_STD  # (N,7,7)
    ty = offset[:, 1][:, part_h[:, None], part_w[None, :]] * TRANS_STD

    wstart = (
        pw.astype(f32)[None, None, :] * bin_w[:, None, None]
        + roi_sw[:, None, None]
        + tx * roi_w[:, None, None]
    )
    hstart = (
        ph.astype(f32)[None, :, None] * bin_h[:, None, None]
        + roi_sh[:, None, None]
        + ty * roi_h[:, None, None]
    )

    iw = np.arange(S, dtype=f32)
    ih = np.arange(S, dtype=f32)
    wpos = (
        wstart[:, :, :, None, None]
        + iw[None, None, None, None, :] * sub_w[:, None, None, None, None]
    )
    hpos = (
        hstart[:, :, :, None, None]
        + ih[None, None, None, :, None] * sub_h[:, None, None, None, None]
    )

    valid = (
        (wpos >= f32(-0.5)) & (wpos <= f32(W) - f32(0.5))
        & (hpos >= f32(-0.5)) & (hpos <= f32(H) - f32(0.5))
    )
    wc = np.clip(wpos, f32(0.0), f32(W - 1.0))
    hc = np.clip(hpos, f32(0.0), f32(H - 1.0))

    x0 = np.floor(wc).astype(np.int32)
    x1 = np.ceil(wc).astype(np.int32)
    y0 = np.floor(hc).astype(np.int32)
    y1 = np.ceil(hc).astype(np.int32)
    dx = (wc - np.floor(wc)).astype(f64)
    dy = (hc - np.floor(hc)).astype(f64)

    cnt = valid.sum(axis=(3, 4)).astype(f32)  # (N,7,7)
    coef = np.where(cnt > 0, 1.0 / np.maximum(cnt, f32(1.0)).astype(f64), 0.0)

    w00 = (1.0 - dx) * (1.0 - dy)
    w01 = dx * (1.0 - dy)
    w10 = (1.0 - dx) * dy
    w11 = dx * dy

    return dict(
        batch=batch, valid=valid, x0=x0, x1=x1, y0=y0, y1=y1,
        w00=w00, w01=w01, w10=w10, w11=w11, coef=coef,
    )


def _build_roi_mats(rois, offset):
    """Per roi: (pixel idx int32 (npix,), A f32 (npix, 49)), npix % 128 == 0."""
    d = _roi_sampling_data(rois, offset)
    j_grid = np.arange(NJ, dtype=np.int64).reshape(P, P, 1, 1)
    j_grid = np.broadcast_to(j_grid, (P, P, S, S))
    full = (P, P, S, S)

    out = []
    for n in range(N_ROIS):
        v = d["valid"][n]
        if not v.any():
            out.append((np.zeros(128, np.int32), np.zeros((128, NJ), f32)))
            continue
        jj = j_grid[v]
        xs0 = np.broadcast_to(d["x0"][n], full)[v]
        xs1 = np.broadcast_to(d["x1"][n], full)[v]
        ys0 = np.broadcast_to(d["y0"][n], full)[v]
        ys1 = np.broadcast_to(d["y1"][n], full)[v]
        cf = np.broadcast_to(d["coef"][n][:, :, None, None], full)[v]
        bx0 = int(xs0.min()); bx1 = int(xs1.max())
        by0 = int(ys0.min()); by1 = int(ys1.max())
        bw = bx1 - bx0 + 1
        bh = by1 - by0 + 1
        npix = bh * bw
        npad = (-npix) % 128
        A = np.zeros((npix + npad, NJ), f64)
        for yy, xx, ww in (
            (ys0, xs0, np.broadcast_to(d["w00"][n], full)[v]),
            (ys0, xs1, np.broadcast_to(d["w01"][n], full)[v]),
            (ys1, xs0, np.broadcast_to(d["w10"][n], full)[v]),
            (ys1, xs1, np.broadcast_to(d["w11"][n], full)[v]),
        ):
            lp = (yy - by0).astype(np.int64) * bw + (xx - bx0)
            np.add.at(A, (lp, jj), ww * cf)
        yidx = (by0 + np.arange(bh, dtype=np.int32))[:, None]
        xidx = (bx0 + np.arange(bw, dtype=np.int32))[None, :]
        gidx = (int(d["batch"][n]) * (H * W) + yidx * W + xidx).reshape(-1)
        gidx = np.concatenate([gidx, np.zeros(npad, np.int32)]).astype(np.int32)
        out.append((gidx, A.astype(f32)))
    return out


# --------------------------------------------------------------------------
# device program
# --------------------------------------------------------------------------
def _build_program(nch):
    """nch: tuple of RPC ints = chunks per roi slot. Same program on 8 cores."""
    import concourse.bacc as bacc
    import concourse.bass as bass
    import concourse.mybir as mybir
    from concourse.tile import TileContext

    T = int(sum(nch))
    col0 = np.concatenate([[0], np.cumsum(nch)]).astype(int)  # slot -> first col

    # split slots into N_GROUPS groups; group 0 small so the pipeline
    # starts early, last groups small so the tail drains fast
    weights = GROUP_WEIGHTS[:N_GROUPS]
    cum = np.cumsum(weights) / sum(weights)
    bounds = [0]
    for g in range(N_GROUPS - 1):
        target = T * cum[g]
        s = int(np.searchsorted(col0, target))
        s = min(max(s, bounds[-1] + 1), RPC - (N_GROUPS - 1 - g))
        bounds.append(s)
    bounds.append(RPC)

    nc = bacc.Bacc("TRN2", num_devices=N_CORES)
    dt = mybir.dt
    fcl = nc.dram_tensor("fcl", [B * H * W, C], dt.float16, kind="ExternalInput")
    amat = nc.dram_tensor("amat", [128, T, NJ], dt.float16, kind="ExternalInput")
    # dma_gather index layout: logical idx i lives at [i % 16, i // 16],
    # replicated across the 8 groups of 16 partitions.
    pidx = nc.dram_tensor("pidx", [128, T * 8], dt.int16, kind="ExternalInput")
    outd = nc.dram_tensor("out", [128, RPC, 2, NJ], dt.float16, kind="ExternalOutput")

    with TileContext(nc) as tc:
        with (
            tc.tile_pool(name="main", bufs=1) as mp,
            tc.tile_pool(name="psum", bufs=2, space="PSUM") as pp,
        ):
            idx_t = mp.tile([128, T * 8], dt.int16, tag="idx")
            nc.sync.dma_start(out=idx_t[:], in_=pidx[:])
            if not A_SPLIT:
                a_full = mp.tile([128, T, NJ], dt.float16, tag="amat")
                nc.sync.dma_start(out=a_full[:], in_=amat[:])

            for g in range(N_GROUPS):
                s0, s1 = bounds[g], bounds[g + 1]
                c0, c1 = int(col0[s0]), int(col0[s1])
                ncols = c1 - c0
                if A_SPLIT:
                    a_g = mp.tile([128, ncols, NJ], dt.float16, tag=f"amat{g}")
                    nc.scalar.dma_start(out=a_g[:], in_=amat[:, c0:c1, :])
                p_t = mp.tile([128, ncols, C], dt.float16, tag=f"patch{g}")
                nc.gpsimd.dma_gather(
                    out_ap=p_t[:],
                    in_ap=fcl[:],
                    idxs_ap=idx_t[:, c0 * 8:c1 * 8],
                    num_idxs=ncols * 128,
                    num_idxs_reg=ncols * 128,
                    elem_size=C,
                    single_packet=False,
                )
                ob = mp.tile([128, s1 - s0, 2, NJ], dt.float16, tag=f"outbuf{g}")
                # pack 5 rois (10 roi-halves x 49) per PSUM bank; one DVE
                # copy per bank instead of one per roi-half
                for b0 in range(s0, s1, 5):
                    b1 = min(b0 + 5, s1)
                    nsl = (b1 - b0) * 2
                    pb = pp.tile([128, nsl * NJ], dt.float32, tag="pbank")
                    for r in range(b0, b1):
                        for h in range(2):
                            o = ((r - b0) * 2 + h) * NJ
                            for t in range(nch[r]):
                                c = int(col0[r]) + t
                                rhs = (
                                    a_g[:, c - c0, :] if A_SPLIT
                                    else a_full[:, c, :]
                                )
                                nc.tensor.matmul(
                                    out=pb[:, o:o + NJ],
                                    lhsT=p_t[:, c - c0, h * 128:(h + 1) * 128],
                                    rhs=rhs,
                                    start=(t == 0),
                                    stop=(t == nch[r] - 1),
                                )
                    nc.vector.tensor_copy(
                        out=ob[:, b0 - s0:b1 - s0, :, :], in_=pb[:, :nsl * NJ]
                    )
                # one output DMA per group; the last group drains per-bank
                # via the loop above having filled ob fully
                nc.sync.dma_start(out=outd[:, s0:s1, :, :], in_=ob[:])
    nc.compile()
    return nc


# --------------------------------------------------------------------------
# entry point
# --------------------------------------------------------------------------
def _partition_rois(mats):
    """Snake-deal rois to cores by descending chunk count so every slot r
    holds 8 near-equal-size rois -> per-slot max (nch) is tight."""
    chunks_per = np.array([len(g) // 128 for g, _ in mats])
    order = np.argsort(-chunks_per, kind="stable")
    slots = [[None] * RPC for _ in range(N_CORES)]  # slots[k][r] = roi index
    for i, roi in enumerate(order):
        rnd, pos = divmod(i, N_CORES)
        core = pos if rnd % 2 == 0 else N_CORES - 1 - pos
        slots[core][rnd] = int(roi)
    slots = [np.array(s) for s in slots]
    nch = tuple(
        int(max(chunks_per[slots[k][r]] for k in range(N_CORES))) for r in range(RPC)
    )
    return slots, nch


def kernel(input, rois, offset):
    from concourse.bass_utils import run_bass_kernel_spmd

    input = np.asarray(input, dtype=f32)
    mats = _build_roi_mats(rois, offset)

    fcl = np.ascontiguousarray(
        input.transpose(0, 2, 3, 1).astype(np.float16)
    ).reshape(B * H * W, C)

    slots, nch = _partition_rois(mats)
    T = int(sum(nch))
    col0 = np.concatenate([[0], np.cumsum(nch)]).astype(int)

    key = nch
    if key not in _prog_cache:
        _prog_cache[key] = _build_program(nch)
    nc = _prog_cache[key]

    in_maps = []
    for k in range(N_CORES):
        logical = np.zeros(T * 128, np.int32)
        a_arr = np.zeros((128, T, NJ), np.float16)
        for r in range(RPC):
            gidx, A = mats[slots[k][r]]
            tchunks = len(gidx) // 128
            for t in range(tchunks):
                col = int(col0[r]) + t
                logical[col * 128:(col + 1) * 128] = gidx[t * 128:(t + 1) * 128]
                a_arr[:, col, :] = A[t * 128:(t + 1) * 128, :]
        # wrap-16 + replicate to 128 partitions (see _build_program)
        idx16 = np.tile(logical.astype(np.int16).reshape(-1, 16).T, (8, 1))
        in_maps.append({"fcl": fcl, "amat": a_arr, "pidx": idx16})

    res = run_bass_kernel_spmd(nc, in_maps, core_ids=list(range(N_CORES)))

    out_full = np.empty((N_ROIS, C, P, P), f32)
    for k in range(N_CORES):
        arr = res.results[k]["out"].astype(f32)  # (128, RPC, 2, 49)
        t = arr.transpose(1, 2, 0, 3).reshape(RPC, C, P, P)
        out_full[slots[k]] = t
    return out_full



# revision 3
# speedup vs baseline: 1.1630x; 1.1630x over previous
"""DCNv2 deformable PS-RoI pooling on 8 Trainium2 NeuronCores.

Strategy (RoI-data-parallel, 32 rois per core, slot-capacity template):
  * Host replicates the reference coordinate math exactly in float32 and folds
    bilinear weights, validity masking and the 1/count normalization into a
    per-roi sparse matrix over the roi's exact touched pixel set (not its
    bbox hull).
  * Rois are snake-dealt to cores by touched-pixel count so the r-th roi of
    every core has a near-identical size; slot r gets a shared pixel capacity
    cap_r = max over cores.  Pixels pack contiguously across slots (no
    per-roi 128-padding; only the final chunk pads), so the gather volume is
    ~2.9k pixels/core instead of 5.5k.
  * Device (SPMD, one program, per-core data in DRAM inputs):
      - one indirect-DMA gather per slot-group: patch[:, t, :] holds 128
        pixels on partitions x 256 channels
      - per (chunk, slot) template block: 2 matmuls (channel halves) of
        patch_chunk(128px, 128c)^T @ A_block(128px, 49) accumulating into the
        group's PSUM bank pair with start/stop on the slot's first/last chunk
      - per group: PSUM -> SBUF copies (DVE for half 0, Act for half 1),
        then one DMA to HBM; host undoes the slot permutation.
"""
import numpy as np

f32 = np.float32
f64 = np.float64

B, C, H, W = 8, 256, 64, 64
N_ROIS, P, S = 256, 7, 4
PART = 7
NJ = P * P  # 49
SCALE = f32(1.0 / 16.0)
TRANS_STD = f32(0.1)
N_CORES = 8
RPC = N_ROIS // N_CORES  # rois (slots) per core
CH = 128  # chunk size (partition dim)
# slot-count per psum group (each group = 2 PSUM banks, <= 10 slots)
GROUP_SIZES = (4, 10, 10, 8)
# rank (0 = largest roi) -> slot position; first/last groups get small rois
ALIGN_PAD = 0  # pad slot tails < ALIGN_PAD px to the next chunk boundary

_prog_cache = {}


# --------------------------------------------------------------------------
# host math: exact f32 replication of the reference coordinate computation
# --------------------------------------------------------------------------
def _roi_sampling_data(rois, offset):
    rois = np.asarray(rois, dtype=f32)
    offset = np.asarray(offset, dtype=f32)
    batch = rois[:, 0].astype(np.int32)

    roi_sw = np.round(rois[:, 1]) * SCALE - f32(0.5)
    roi_sh = np.round(rois[:, 2]) * SCALE - f32(0.5)
    roi_ew = (np.round(rois[:, 3]) + f32(1.0)) * SCALE - f32(0.5)
    roi_eh = (np.round(rois[:, 4]) + f32(1.0)) * SCALE - f32(0.5)
    roi_w = np.maximum(roi_ew - roi_sw, f32(0.1))
    roi_h = np.maximum(roi_eh - roi_sh, f32(0.1))
    bin_w = roi_w / f32(P)
    bin_h = roi_h / f32(P)
    sub_w = bin_w / f32(S)
    sub_h = bin_h / f32(S)

    ph = np.arange(P, dtype=np.int32)
    pw = np.arange(P, dtype=np.int32)
    part_h = np.clip(
        np.floor(ph.astype(f32) / f32(P) * f32(PART)).astype(np.int32), 0, PART - 1
    )
    part_w = np.clip(
        np.floor(pw.astype(f32) / f32(P) * f32(PART)).astype(np.int32), 0, PART - 1
    )

    tx = offset[:, 0][:, part_h[:, None], part_w[None, :]] * TRANS_STD  # (N,7,7)
    ty = offset[:, 1][:, part_h[:, None], part_w[None, :]] * TRANS_STD

    wstart = (
        pw.astype(f32)[None, None, :] * bin_w[:, None, None]
        + roi_sw[:, None, None]
        + tx * roi_w[:, None, None]
    )
    hstart = (
        ph.astype(f32)[None, :, None] * bin_h[:, None, None]
        + roi_sh[:, None, None]
        + ty * roi_h[:, None, None]
    )

    iw = np.arange(S, dtype=f32)
    ih = np.arange(S, dtype=f32)
    wpos = (
        wstart[:, :, :, None, None]
        + iw[None, None, None, None, :] * sub_w[:, None, None, None, None]
    )
    hpos = (
        hstart[:, :, :, None, None]
        + ih[None, None, None, :, None] * sub_h[:, None, None, None, None]
    )

    valid = (
        (wpos >= f32(-0.5)) & (wpos <= f32(W) - f32(0.5))
        & (hpos >= f32(-0.5)) & (hpos <= f32(H) - f32(0.5))
    )
    wc = np.clip(wpos, f32(0.0), f32(W - 1.0))
    hc = np.clip(hpos, f32(0.0), f32(H - 1.0))

    x0 = np.floor(wc).astype(np.int32)
    x1 = np.ceil(wc).astype(np.int32)
    y0 = np.floor(hc).astype(np.int32)
    y1 = np.ceil(hc).astype(np.int32)
    dx = (wc - np.floor(wc)).astype(f64)
    dy = (hc - np.floor(hc)).astype(f64)

    cnt = valid.sum(axis=(3, 4)).astype(f32)  # (N,7,7)
    coef = np.where(cnt > 0, 1.0 / np.maximum(cnt, f32(1.0)).astype(f64), 0.0)

    w00 = (1.0 - dx) * (1.0 - dy)
    w01 = dx * (1.0 - dy)
    w10 = (1.0 - dx) * dy
    w11 = dx * dy

    return dict(
        batch=batch, valid=valid, x0=x0, x1=x1, y0=y0, y1=y1,
        w00=w00, w01=w01, w10=w10, w11=w11, coef=coef,
    )


def _build_roi_sparse(rois, offset):
    """Per roi: (sorted unique pixel ids (npix,), A f64 (npix, 49))."""
    d = _roi_sampling_data(rois, offset)
    full = (P, P, S, S)
    j_grid = np.broadcast_to(
        np.arange(NJ, dtype=np.int64).reshape(P, P, 1, 1), full
    )

    out = []
    for n in range(N_ROIS):
        v = d["valid"][n]
        if not v.any():
            out.append((np.zeros(0, np.int32), np.zeros((0, NJ), f64)))
            continue
        jj = j_grid[v]
        b = int(d["batch"][n])
        ids = []
        ws = []
        for xk, yk, wk in (
            ("x0", "y0", "w00"), ("x1", "y0", "w01"),
            ("x0", "y1", "w10"), ("x1", "y1", "w11"),
        ):
            xs = np.broadcast_to(d[xk][n], full)[v].astype(np.int64)
            ys = np.broadcast_to(d[yk][n], full)[v].astype(np.int64)
            cf = np.broadcast_to(d["coef"][n][:, :, None, None], full)[v]
            ids.append(b * (H * W) + ys * W + xs)
            ws.append(np.broadcast_to(d[wk][n], full)[v] * cf)
        ids = np.concatenate(ids)
        ws = np.concatenate(ws)
        jjs = np.concatenate([jj] * 4)
        uniq, inv = np.unique(ids, return_inverse=True)
        A = np.zeros((len(uniq), NJ), f64)
        np.add.at(A, (inv, jjs), ws)
        out.append((uniq.astype(np.int32), A))
    return out


# --------------------------------------------------------------------------
# planning: snake deal -> slot capacities -> chunk/block template
# --------------------------------------------------------------------------
def _plan(sizes):
    """sizes: (256,) touched-pixel counts. Returns slots (core->slot->roi),
    caps, and the template (chunks, blocks, groups)."""
    order = np.argsort(-sizes, kind="stable")
    ranks = [[None] * RPC for _ in range(N_CORES)]  # ranks[k][rank] = roi
    for i, roi in enumerate(order):
        rnd, pos = divmod(i, N_CORES)
        core = pos if rnd % 2 == 0 else N_CORES - 1 - pos
        ranks[core][rnd] = int(roi)

    # rank -> slot position: first group small, big rois in middle groups,
    # smallest rois in the last group
    n0, n1, n2, n3 = GROUP_SIZES
    slot_rank = (
        list(range(n1 + n2, n1 + n2 + n0))         # g0: mid-small ranks
        + list(range(0, n1))                        # g1: biggest
        + list(range(n1, n1 + n2))                  # g2: next
        + list(range(n1 + n2 + n0, RPC))            # g3: smallest
    )
    assert sorted(slot_rank) == list(range(RPC))
    slots = [[ranks[k][slot_rank[r]] for r in range(RPC)] for k in range(N_CORES)]

    caps = []
    for r in range(RPC):
        cap = max(int(sizes[slots[k][r]]) for k in range(N_CORES))
        caps.append(cap)

    # optional alignment padding: if a slot's tail crosses a chunk boundary
    # by < ALIGN_PAD px, round the running offset up to the boundary
    pos = []
    off = 0
    for r in range(RPC):
        pos.append(off)
        off += caps[r]
        if ALIGN_PAD and r + 1 < RPC:
            tail = off % CH
            if 0 < tail < ALIGN_PAD:
                off += CH - tail  # next slot starts at a chunk boundary
    total = off
    T = -(-total // CH)

    # blocks: (t, slot) incidences in (group, t, slot) order
    gb = np.cumsum((0,) + GROUP_SIZES)
    groups = []
    aoff = 0
    for g in range(len(GROUP_SIZES)):
        s0, s1 = int(gb[g]), int(gb[g + 1])
        blocks = []
        for r in range(s0, s1):
            if caps[r] == 0:
                continue
            t_first = pos[r] // CH
            t_last = (pos[r] + caps[r] - 1) // CH
            for t in range(t_first, t_last + 1):
                blocks.append((t, r, aoff, t == t_first, t == t_last))
                aoff += NJ
        blocks.sort(key=lambda b: (b[0], b[1]))
        # gather chunk range for this group (dedup: chunk owned by first group)
        groups.append((s0, s1, blocks))
    acols = aoff

    # gather ranges: group g gathers chunks [gt0, gt1)
    granges = []
    t_done = 0
    for g, (s0, s1, blocks) in enumerate(groups):
        if g == len(GROUP_SIZES) - 1:
            t_hi = T
        else:
            last = pos[s1 - 1] + caps[s1 - 1] - 1
            t_hi = last // CH + 1
        granges.append((t_done, max(t_hi, t_done)))
        t_done = max(t_hi, t_done)
    assert t_done == T

    return dict(slots=slots, caps=caps, pos=pos, T=T, acols=acols,
                groups=groups, granges=granges)


def _plan_key(plan):
    key = (tuple(plan["caps"]), plan["T"], plan["acols"], tuple(plan["granges"]))
    blocks_key = tuple(
        (s0, s1, tuple(b)) for (s0, s1, blks) in plan["groups"] for b in blks
    )
    return key + (blocks_key,)


# --------------------------------------------------------------------------
# device program
# --------------------------------------------------------------------------
def _build_program(plan):
    import concourse.bacc as bacc
    import concourse.mybir as mybir
    from concourse.tile import TileContext

    T = plan["T"]
    acols = plan["acols"]
    groups = plan["groups"]
    granges = plan["granges"]

    nc = bacc.Bacc("TRN2", num_devices=N_CORES)
    dt = mybir.dt
    fcl = nc.dram_tensor("fcl", [B * H * W, C], dt.float16, kind="ExternalInput")
    amat = nc.dram_tensor("amat", [128, acols], dt.float16, kind="ExternalInput")
    pidx = nc.dram_tensor("pidx", [128, T * 8], dt.int16, kind="ExternalInput")
    outd = nc.dram_tensor("out", [128, RPC, 2, NJ], dt.float16, kind="ExternalOutput")

    # split the A upload so the first groups' blocks arrive early
    a_split = 0
    for g, (s0, s1, blocks) in enumerate(groups[:2]):
        if blocks:
            a_split = max(a_split, max(b[2] for b in blocks) + NJ)
    a_split = min(a_split, acols)

    with TileContext(nc) as tc:
        with (
            tc.tile_pool(name="main", bufs=1) as mp,
            tc.tile_pool(name="psum", bufs=1, space="PSUM") as pp,
        ):
            idx_t = mp.tile([128, T * 8], dt.int16, tag="idx")
            nc.sync.dma_start(out=idx_t[:], in_=pidx[:])
            a_t = mp.tile([128, acols], dt.float16, tag="amat")
            nc.sync.dma_start(out=a_t[:, :a_split], in_=amat[:, :a_split])
            if a_split < acols:
                nc.sync.dma_start(out=a_t[:, a_split:], in_=amat[:, a_split:])
            patch = mp.tile([128, T, C], dt.float16, tag="patch")

            for g, (t0, t1) in enumerate(granges):
                if t1 <= t0:
                    continue
                nc.gpsimd.dma_gather(
                    out_ap=patch[:, t0:t1, :],
                    in_ap=fcl[:],
                    idxs_ap=idx_t[:, t0 * 8:t1 * 8],
                    num_idxs=(t1 - t0) * 128,
                    num_idxs_reg=(t1 - t0) * 128,
                    elem_size=C,
                    single_packet=False,
                )

            for g, (s0, s1, blocks) in enumerate(groups):
                ns = s1 - s0
                pbs = [
                    pp.tile([128, ns, NJ], dt.float32, tag=f"pb{g}_{h}",
                            name=f"pb{g}_{h}")
                    for h in range(2)
                ]
                for (t, r, aoff, st, sp) in blocks:
                    for h in range(2):
                        nc.tensor.matmul(
                            out=pbs[h][:, r - s0, :],
                            lhsT=patch[:, t, h * 128:(h + 1) * 128],
                            rhs=a_t[:, aoff:aoff + NJ],
                            start=st,
                            stop=sp,
                        )
                ob = mp.tile([128, ns, 2, NJ], dt.float16, tag=f"ob{g}")
                nc.vector.tensor_copy(out=ob[:, :, 0, :], in_=pbs[0][:])
                nc.scalar.copy(out=ob[:, :, 1, :], in_=pbs[1][:])
                nc.sync.dma_start(out=outd[:, s0:s1, :, :], in_=ob[:])
    nc.compile()
    return nc


# --------------------------------------------------------------------------
# entry point
# --------------------------------------------------------------------------
def kernel(input, rois, offset):
    from concourse.bass_utils import run_bass_kernel_spmd

    input = np.asarray(input, dtype=f32)
    mats = _build_roi_sparse(rois, offset)
    sizes = np.array([len(g) for g, _ in mats])
    plan = _plan(sizes)

    key = _plan_key(plan)
    if key not in _prog_cache:
        _prog_cache[key] = _build_program(plan)
    nc = _prog_cache[key]

    fcl = np.ascontiguousarray(
        input.transpose(0, 2, 3, 1).astype(np.float16)
    ).reshape(B * H * W, C)

    T, acols = plan["T"], plan["acols"]
    caps, pos, slots = plan["caps"], plan["pos"], plan["slots"]

    in_maps = []
    for k in range(N_CORES):
        logical = np.zeros(T * CH, np.int32)
        a_arr = np.zeros((128, acols), np.float16)
        for (s0, s1, blocks) in plan["groups"]:
            for (t, r, aoff, st, sp) in blocks:
                gidx, A = mats[slots[k][r]]
                npix = len(gidx)
                lo = max(t * CH, pos[r])
                hi = min((t + 1) * CH, pos[r] + npix)
                if hi <= lo:
                    continue
                i0 = lo - pos[r]
                i1 = hi - pos[r]
                logical[lo:hi] = gidx[i0:i1]
                a_arr[lo - t * CH:hi - t * CH, aoff:aoff + NJ] = (
                    A[i0:i1].astype(np.float16)
                )
        idx16 = np.tile(logical.astype(np.int16).reshape(-1, 16).T, (8, 1))
        in_maps.append({"fcl": fcl, "amat": a_arr, "pidx": idx16})

    res = run_bass_kernel_spmd(nc, in_maps, core_ids=list(range(N_CORES)))

    out_full = np.empty((N_ROIS, C, P, P), f32)
    for k in range(N_CORES):
        arr = res.results[k]["out"].astype(f32)  # (128, RPC, 2, 49)
        t = arr.transpose(1, 2, 0, 3).reshape(RPC, C, P, P)
        for r in range(RPC):
            roi = slots[k][r]
            if len(mats[roi][0]) == 0 and caps[r] > 0:
                out_full[roi] = 0.0
            elif caps[r] == 0:
                out_full[roi] = 0.0
            else:
                out_full[roi] = t[r]
    return out_full
